# revision 1
# baseline (speedup 1.0000x reference)
"""Trainium2 Bass kernel v3 for nn_EnhancedLossModule.

Per-core plan (8 cores, 256 rows each):
  - Pair scalars (r_i, r_p, <fi,fp>) via PE matmuls on transposed pair
    features + identity-mask diagonal extraction (runs first, PE warms up).
  - G = f_loc @ f_all^T via bf16 PE matmuls (PSUM fp32), double-buffered.
  - r (row norms) via squares of featT tiles + gpsimd partition_all_reduce.
  - Dense passes in bf16/fp16 SBUF (2x/4x DVE modes), labels as int16.
  - Distance rows stored fp16 to DRAM; same-label pairs packed into anchor
    slots (J=4 partners/slot) so each anchor row is gathered once per slot.
  - tensor_scalar accum trick: sum min(x, c) reduces at 4x rate; the
    -N*c correction is applied on the host / in a tiny column op.
  - Focal/label-smoothing data-parallel on the pred shard (bf16).
  - All ACT Exp/Ln ops are delayed (tile_wait_until) so the activation
    table only switches sqrt->exp once.
  - Host sums the per-core [128, NCOL] accumulators and combines losses.
"""

import math

import ml_dtypes
import numpy as np

import concourse.bacc as bacc
import concourse.bass as bass
import concourse.bass_isa as bass_isa
import concourse.tile as tile
from concourse import mybir
from concourse.bass_utils import run_bass_kernel_spmd

B, C, D = 2048, 1000, 512
N_CORES = 8
R = B // N_CORES            # 256 rows per core
RT = R // 128               # 2 row tiles
KT = D // 128               # 4 contraction tiles
J = 4                       # pair partners per anchor slot

TEMPERATURE = 0.07
C_MARGIN = 0.5
T_MARGIN = 1.0
GAMMA = 2.0
ALPHA = 0.25
SMOOTHING = 0.1
W_CONTRASTIVE = 0.1
W_TRIPLET = 0.1
W_FOCAL = 0.4
W_LABEL_SMOOTH = 0.4

MASK = 4096.0               # added to d^2 of same-label entries
XPAD = 65536.0              # invalid-slot x offset
OFF = SMOOTHING / (C - 1)
import os
EXPWAIT_MS = float(os.environ.get("EXPWAIT_MS", "0.028"))

F32 = mybir.dt.float32
BF16 = mybir.dt.bfloat16
FP16 = mybir.dt.float16
I16 = mybir.dt.int16
ALU = mybir.AluOpType
AF = mybir.ActivationFunctionType

_BUILD_CACHE: dict = {}


def _build(T0: int, T1: int):
    """T0/T1 = anchor-slot tiles whose anchors live in row-tile 0/1."""
    key = (T0, T1)
    if key in _BUILD_CACHE:
        return _BUILD_CACHE[key]
    T = T0 + T1
    G = T * J                   # pair-column groups
    TS = [T0, T1]
    SECT = 128 * (1 + 2 * J)    # psum section width per pair tile

    # accumulator columns
    COL_NEG = 0                 # 2: sum min(sim, 0.5) per row tile
    COL_SELF = 2                # 2: sum min(d', margin) per row tile
    COL_PAIR = 4                # G: sum min(d'row, x) - B*x per (t,j)
    COL_POS = COL_PAIR + G      # 1: sum valid * -ln(exp(simp/T)+1e-8)
    COL_NCO = COL_POS + 1       # 1: sum valid * min(simp, 0.5)
    COL_FOC = COL_NCO + 1       # 2: focal per row tile
    COL_LS = COL_FOC + 2        # 2: label smoothing per row tile
    NCOL = COL_LS + 2

    nc = bacc.Bacc("TRN2", target_bir_lowering=False, debug=False,
                   num_devices=N_CORES)

    # ---- DRAM I/O ----
    featT = nc.dram_tensor("featT", [D, B], BF16, kind="ExternalInput")
    ftl2 = nc.dram_tensor("ftl2", [128, KT * R], BF16, kind="ExternalInput")
    featl2 = nc.dram_tensor("featl2", [128, RT * D], BF16,
                            kind="ExternalInput")
    pred2 = nc.dram_tensor("pred2", [128, RT * C], BF16,
                           kind="ExternalInput")
    lab_row = nc.dram_tensor("lab_row", [1, B], I16, kind="ExternalInput")
    lab_loc2 = nc.dram_tensor("lab_loc2", [128, RT], F32,
                              kind="ExternalInput")
    pfiT = nc.dram_tensor("pfiT", [128, KT * T * 128], BF16,
                          kind="ExternalInput")
    pfpT = nc.dram_tensor("pfpT", [128, KT * G * 128], BF16,
                          kind="ExternalInput")
    pidx = nc.dram_tensor("pidx", [128, T * 8], I16, kind="ExternalInput")
    pval = nc.dram_tensor("pval", [128, G], F32, kind="ExternalInput")
    acc_out = nc.dram_tensor("acc_out", [128, NCOL], F32,
                             kind="ExternalOutput")

    def bcast_ap(handle, n):
        a = handle.ap()
        return bass.AP(tensor=a.tensor, offset=a.offset,
                       ap=[[0, 128], [1, n]])

    with tile.TileContext(nc) as tc:
        with (
            tc.tile_pool(name="persist", bufs=1) as persist,
            tc.tile_pool(name="scratch", bufs=2) as scratch,
            tc.tile_pool(name="small", bufs=2) as small,
            tc.tile_pool(name="gpsum", bufs=2, space="PSUM") as gpsum,
            tc.tile_pool(name="dscratch", bufs=1, space="DRAM") as dscratch,
        ):
            dp_dram = [dscratch.tile([128, B], FP16, tag=f"dp{m}",
                                     name=f"dp{m}")
                       for m in range(RT)]

            # ---------------- constants ----------------
            iota_c = persist.tile([128, C], I16)
            nc.gpsimd.iota(iota_c, pattern=[[1, C]], base=0,
                           channel_multiplier=0,
                           allow_small_or_imprecise_dtypes=True)
            iota_sq = persist.tile([128, 128], F32)
            nc.gpsimd.iota(iota_sq, pattern=[[1, 128]], base=0,
                           channel_multiplier=0,
                           allow_small_or_imprecise_dtypes=True)
            pid = persist.tile([128, 1], F32)
            nc.gpsimd.iota(pid, pattern=[[0, 1]], base=0,
                           channel_multiplier=1,
                           allow_small_or_imprecise_dtypes=True)
            ident = persist.tile([128, 128], BF16)
            nc.vector.tensor_scalar(out=ident, in0=iota_sq, scalar1=pid,
                                    scalar2=None, op0=ALU.is_equal)

            # ---------------- inputs ----------------
            # featT tiles first; pair features last (needed mid-kernel)
            ft = []
            dmaq = [nc.sync, nc.scalar]
            for k in range(KT):
                t = persist.tile([128, B], BF16, tag=f"ft{k}")
                dmaq[k % 2].dma_start(
                    out=t, in_=featT.ap()[k * 128:(k + 1) * 128, :])
                ft.append(t)
            ftl = persist.tile([128, KT * R], BF16)
            nc.sync.dma_start(out=ftl, in_=ftl2.ap())
            fl2 = persist.tile([128, RT * D], BF16)
            nc.scalar.dma_start(out=fl2, in_=featl2.ap())
            lab_r = persist.tile([1, B], I16)
            nc.scalar.dma_start(out=lab_r, in_=lab_row.ap())
            lab_b = persist.tile([128, B], I16)
            nc.gpsimd.partition_broadcast(lab_b, lab_r, channels=128)
            labl = persist.tile([128, RT], F32)
            nc.scalar.dma_start(out=labl, in_=lab_loc2.ap())
            idx_sb = persist.tile([128, T * 8], I16)
            nc.scalar.dma_start(out=idx_sb, in_=pidx.ap())
            pval_t = persist.tile([128, G], F32)
            nc.scalar.dma_start(out=pval_t, in_=pval.ap())
            fiT = persist.tile([128, KT * T * 128], BF16)
            nc.sync.dma_start(out=fiT, in_=pfiT.ap())
            pr2 = persist.tile([128, RT * C], BF16)
            nc.scalar.dma_start(out=pr2, in_=pred2.ap())
            fpT = persist.tile([128, KT * G * 128], BF16)
            nc.scalar.dma_start(out=fpT, in_=pfpT.ap())
            # junk sinks (one per engine so WAR chains stay intra-engine)
            junk_v = persist.tile([128, B], FP16)
            junk_a = persist.tile([128, B], BF16)
            junk_p = persist.tile([128, C], BF16)
            acc = persist.tile([128, NCOL], F32)
            nc.vector.memset(acc, 0.0)

            # ---------------- r row norms ----------------
            sq = []
            for k in range(KT):
                s = scratch.tile([128, B], BF16, tag=f"sq{k % 2}",
                                 name=f"sq{k}")
                nc.vector.tensor_mul(s, ft[k], ft[k])
                sq.append(s)
            ones_col = persist.tile([128, 1], BF16)
            nc.gpsimd.memset(ones_col, 1.0)
            rps = gpsum.tile([128, B], F32, tag="big", name="rps")
            for nch in range(4):
                for k in range(KT):
                    nc.tensor.matmul(
                        rps[0:1, nch * 512:(nch + 1) * 512],
                        ones_col,
                        sq[k][:, nch * 512:(nch + 1) * 512],
                        start=(k == 0), stop=(k == KT - 1),
                    )
            r_row = persist.tile([1, B], BF16)
            nc.scalar.activation(out=r_row, in_=rps[0:1, :], func=AF.Copy)
            r_b = persist.tile([128, B], BF16)
            nc.gpsimd.partition_broadcast(r_b, r_row, channels=128)
            rho_b = persist.tile([128, B], BF16)
            nc.scalar.activation(out=rho_b, in_=r_b, func=AF.Sqrt)
            s_b = persist.tile([128, B], BF16)
            with nc.allow_low_precision(reason="1/rho in bf16; sim tolerance"):
                nc.vector.reciprocal(out=s_b, in_=rho_b)

            # local row norms: rloc[:, m] = sum(f_loc_m^2) (fp32)
            rloc = persist.tile([128, RT], F32)
            for m in range(RT):
                nc.scalar.activation(
                    out=junk_a[:, 0:D], in_=fl2[:, m * D:(m + 1) * D],
                    func=AF.Square, accum_out=rloc[:, m:m + 1])
            rho_loc = persist.tile([128, RT], F32)
            nc.scalar.activation(out=rho_loc, in_=rloc, func=AF.Sqrt)
            s_loc = persist.tile([128, RT], F32)
            nc.vector.reciprocal(out=s_loc, in_=rho_loc)
            nhs = persist.tile([128, RT], F32)
            nc.vector.tensor_scalar(out=nhs, in0=s_loc, scalar1=-0.5,
                                    scalar2=None, op0=ALU.mult)

            # ---------------- dense: G matmuls + passes ----------------
            for m in range(RT):
                gps = gpsum.tile([128, B], F32, tag="big", name=f"gps{m}")
                for nch in range(4):
                    for k in range(KT):
                        nc.tensor.matmul(
                            gps[:, nch * 512:(nch + 1) * 512],
                            ftl[:, k * R + m * 128:k * R + (m + 1) * 128],
                            ft[k][:, nch * 512:(nch + 1) * 512],
                            start=(k == 0), stop=(k == KT - 1),
                        )
                # Gfp = -2*G + r_i (fold r_i into the copy)       [ACT]
                gfp = scratch.tile([128, B], FP16, tag="gfp")
                nc.scalar.activation(out=gfp, in_=gps, func=AF.Identity,
                                     scale=-2.0, bias=rloc[:, m:m + 1])
                # P1: mwork = (lab_b == lab_i) * MASK             [DVE 4x]
                mwork = scratch.tile([128, B], BF16, tag="mwork")
                nc.vector.tensor_scalar(out=mwork, in0=lab_b,
                                        scalar1=labl[:, m:m + 1],
                                        scalar2=MASK,
                                        op0=ALU.is_equal, op1=ALU.mult)
                # P2: rbl = mwork + r_b                           [DVE 2x]
                rbl = scratch.tile([128, B], BF16, tag="rbl")
                nc.vector.tensor_add(rbl, mwork, r_b)
                # P3: d2 = gfp + rbl                              [DVE 2x]
                d2 = scratch.tile([128, B], FP16, tag="d2")
                nc.vector.tensor_add(d2, gfp, rbl)
                # P4: dpt = sqrt(d2)                              [ACT]
                dpt = scratch.tile([128, B], FP16, tag="dpt")
                nc.scalar.activation(out=dpt, in_=d2, func=AF.Sqrt)
                # P5: store row-tile of d' to DRAM                [DMA]
                (nc.scalar if m == 0 else nc.sync).dma_start(
                    out=dp_dram[m][:, :], in_=dpt)
                # P6: accumulate sum min(d', margin)              [DVE 4x]
                nc.vector.tensor_scalar(
                    out=junk_v, in0=dpt, scalar1=float(T_MARGIN),
                    scalar2=None, op0=ALU.min, op1=ALU.add,
                    accum_out=acc[:, COL_SELF + m:COL_SELF + m + 1])
                # P7a: v = (gfp - r_i) * (-s_i/2) = G*s_i        [DVE 4x]
                vsim = scratch.tile([128, B], FP16, tag="vsim")
                nc.vector.tensor_scalar(
                    out=vsim, in0=gfp, scalar1=rloc[:, m:m + 1],
                    scalar2=nhs[:, m:m + 1],
                    op0=ALU.subtract, op1=ALU.mult)
                # P7b: sim = v * (1/rho_j)                        [DVE 2x]
                sim = scratch.tile([128, B], BF16, tag="sim")
                nc.vector.tensor_mul(sim, vsim, s_b)
                # P8: accumulate sum min(sim, 0.5)                [DVE 4x]
                nc.vector.tensor_scalar(
                    out=junk_v, in0=sim, scalar1=float(C_MARGIN),
                    scalar2=None, op0=ALU.min, op1=ALU.add,
                    accum_out=acc[:, COL_NEG + m:COL_NEG + m + 1])

            # ---------------- pair scalars via PE (PSUM recycled after dense) ----------------
            pri_t = persist.tile([128, T], F32)
            pg = persist.tile([128, G], F32)
            prp = persist.tile([128, G], F32)
            for t in range(T):
                pps = gpsum.tile([128, SECT], F32, tag="big",
                                 name=f"pps{t}")
                fi_sl = [fiT[:, k * (T * 128) + t * 128:
                             k * (T * 128) + (t + 1) * 128]
                         for k in range(KT)]
                for k in range(KT):
                    nc.tensor.matmul(pps[:, 0:128],
                                     fi_sl[k], fi_sl[k],
                                     start=(k == 0), stop=(k == KT - 1))
                for j in range(J):
                    g = t * J + j
                    fp_sl = [fpT[:, k * (G * 128) + g * 128:
                                 k * (G * 128) + (g + 1) * 128]
                             for k in range(KT)]
                    o1 = 128 + j * 256
                    for k in range(KT):
                        nc.tensor.matmul(pps[:, o1:o1 + 128],
                                         fi_sl[k], fp_sl[k],
                                         start=(k == 0), stop=(k == KT - 1))
                    for k in range(KT):
                        nc.tensor.matmul(pps[:, o1 + 128:o1 + 256],
                                         fp_sl[k], fp_sl[k],
                                         start=(k == 0), stop=(k == KT - 1))
                # diagonal extraction (DVE; PSUM src, 128-wide)
                nc.vector.scalar_tensor_tensor(
                    out=junk_v[:, 0:128], in0=pps[:, 0:128],
                    scalar=1.0, in1=ident, op0=ALU.mult, op1=ALU.mult,
                    accum_out=pri_t[:, t:t + 1])
                for j in range(J):
                    g = t * J + j
                    o1 = 128 + j * 256
                    nc.vector.scalar_tensor_tensor(
                        out=junk_v[:, 0:128], in0=pps[:, o1:o1 + 128],
                        scalar=1.0, in1=ident, op0=ALU.mult, op1=ALU.mult,
                        accum_out=pg[:, g:g + 1])
                    nc.vector.scalar_tensor_tensor(
                        out=junk_v[:, 0:128], in0=pps[:, o1 + 128:o1 + 256],
                        scalar=1.0, in1=ident, op0=ALU.mult, op1=ALU.mult,
                        accum_out=prp[:, g:g + 1])

            # pri expanded to [128, G] in (t, j) order via stride-0 AP
            pa = pri_t[:, 0:T]
            pri_g = bass.AP(tensor=pa.tensor, offset=pa.offset,
                            ap=[pa.ap[0], [pa.ap[1][0], T], [0, J]])

            # ---- pair tiny column math [128, G] fp32 ----
            rs = small.tile([128, G], F32, tag="rs")
            nc.vector.tensor_add(rs, pri_g, prp)
            d2p = small.tile([128, G], F32, tag="d2p")
            nc.vector.scalar_tensor_tensor(out=d2p, in0=pg, scalar=-2.0,
                                           in1=rs, op0=ALU.mult,
                                           op1=ALU.add)
            d2rr = small.tile([128, 2 * G], F32, tag="d2rr")
            nc.vector.tensor_scalar(out=d2rr[:, 0:G], in0=d2p, scalar1=0.0,
                                    scalar2=None, op0=ALU.max)
            nc.vector.tensor_mul(d2rr[:, G:2 * G], pri_g, prp)
            dro = small.tile([128, 2 * G], F32, tag="dro")
            nc.scalar.activation(out=dro, in_=d2rr, func=AF.Sqrt)
            dpair = dro[:, 0:G]
            rro = dro[:, G:2 * G]
            xtmp = small.tile([128, G], F32, tag="xtmp")
            nc.vector.scalar_tensor_tensor(
                out=xtmp, in0=dpair, scalar=float(T_MARGIN + XPAD),
                in1=pval_t, op0=ALU.add, op1=ALU.mult)
            xcol = persist.tile([128, G], F32)
            nc.vector.tensor_scalar(out=xcol, in0=xtmp,
                                    scalar1=float(-XPAD), scalar2=None,
                                    op0=ALU.add)
            num = small.tile([128, G], F32, tag="num")
            nc.vector.tensor_sub(num, rs, d2p)
            irr = small.tile([128, G], F32, tag="irr")
            nc.vector.reciprocal(out=irr, in_=rro)
            simp = small.tile([128, G], F32, tag="simp")
            nc.vector.scalar_tensor_tensor(out=simp, in0=num, scalar=0.5,
                                           in1=irr, op0=ALU.mult,
                                           op1=ALU.mult)
            # pos: -ln(exp(simp/T)+1e-8) ~= -simp/T (exp >> 1e-8 here)
            nc.vector.scalar_tensor_tensor(
                out=junk_v[:, 0:G], in0=simp,
                scalar=float(-1.0 / TEMPERATURE), in1=pval_t,
                op0=ALU.mult, op1=ALU.mult,
                accum_out=acc[:, COL_POS:COL_POS + 1])
            nc.vector.scalar_tensor_tensor(
                out=junk_v[:, 0:G], in0=simp, scalar=0.5, in1=pval_t,
                op0=ALU.min, op1=ALU.mult,
                accum_out=acc[:, COL_NCO:COL_NCO + 1])

            # ---------------- focal + label smoothing ----------------
            se = small.tile([128, RT], F32, tag="se")
            spred = small.tile([128, RT], F32, tag="spred")
            ptgt = small.tile([128, RT], F32, tag="ptgt")
            for m in range(RT):
                prm = pr2[:, m * C:(m + 1) * C]
                nc.vector.tensor_scalar(out=junk_v[:, 0:C], in0=prm,
                                        scalar1=1.0, scalar2=None,
                                        op0=ALU.mult, op1=ALU.add,
                                        accum_out=spred[:, m:m + 1])
                tmask = scratch.tile([128, C], BF16, tag="tmask")
                nc.vector.tensor_scalar(out=tmask, in0=iota_c,
                                        scalar1=labl[:, m:m + 1],
                                        scalar2=None, op0=ALU.is_equal)
                nc.vector.scalar_tensor_tensor(
                    out=junk_v[:, 0:C], in0=prm, scalar=1.0, in1=tmask,
                    op0=ALU.mult, op1=ALU.mult,
                    accum_out=ptgt[:, m:m + 1])

            # ---- delayed exp/ln block (single table switch) ----
            with tc.tile_wait_until(EXPWAIT_MS):
                for m in range(RT):
                    prm = pr2[:, m * C:(m + 1) * C]
                    nc.scalar.activation(out=junk_a[:, 0:C], in_=prm,
                                         func=AF.Exp,
                                         accum_out=se[:, m:m + 1])
                eptgt = small.tile([128, RT], F32, tag="eptgt")
                nc.scalar.activation(out=eptgt, in_=ptgt, func=AF.Exp)
                lse = small.tile([128, RT], F32, tag="lse")
                nc.scalar.activation(out=lse, in_=se, func=AF.Ln)
                ce = small.tile([128, RT], F32, tag="ce")
                nc.vector.tensor_sub(ce, lse, ptgt)
                invse = small.tile([128, RT], F32, tag="invse")
                nc.vector.reciprocal(out=invse, in_=se)
                pt = small.tile([128, RT], F32, tag="pt")
                nc.vector.tensor_mul(pt, eptgt, invse)
                onept = small.tile([128, RT], F32, tag="onept")
                nc.vector.tensor_scalar(out=onept, in0=pt, scalar1=-1.0,
                                        scalar2=1.0, op0=ALU.mult,
                                        op1=ALU.add)
                f2 = small.tile([128, RT], F32, tag="f2")
                nc.vector.tensor_mul(f2, onept, onept)
                nc.vector.tensor_mul(
                    acc[:, COL_FOC:COL_FOC + RT], f2, ce)
                t1 = small.tile([128, RT], F32, tag="t1")
                nc.vector.tensor_scalar(out=t1, in0=spred,
                                        scalar1=float(-OFF), scalar2=None,
                                        op0=ALU.mult)
                t2 = small.tile([128, RT], F32, tag="t2")
                nc.vector.scalar_tensor_tensor(
                    out=t2, in0=ptgt,
                    scalar=float(-(1.0 - SMOOTHING - OFF)), in1=t1,
                    op0=ALU.mult, op1=ALU.add)
                nc.vector.tensor_add(
                    acc[:, COL_LS:COL_LS + RT], lse, t2)



            # ---------------- gather + pair reductions ----------------
            # sum min(grow, x) per slot; then acc[PAIR] = pracc - B*x
            pracc = persist.tile([128, G], F32)
            toff = 0
            for m in range(RT):
                Tm = TS[m]
                if Tm == 0:
                    continue
                grow = persist.tile([128, Tm, B], FP16, tag=f"grow{m}",
                                    name=f"grow{m}")
                nc.gpsimd.dma_gather(
                    out_ap=grow,
                    in_ap=dp_dram[m][:, :],
                    idxs_ap=idx_sb[:, toff * 8:(toff + Tm) * 8],
                    num_idxs=Tm * 128,
                    num_idxs_reg=Tm * 128,
                    elem_size=B,
                )
                for tl in range(Tm):
                    for j in range(J):
                        g = (toff + tl) * J + j
                        nc.vector.tensor_scalar(
                            out=junk_v, in0=grow[:, tl, :],
                            scalar1=xcol[:, g:g + 1], scalar2=None,
                            op0=ALU.min, op1=ALU.add,
                            accum_out=pracc[:, g:g + 1])
                toff += Tm
            nc.vector.scalar_tensor_tensor(
                out=acc[:, COL_PAIR:COL_PAIR + G], in0=xcol,
                scalar=float(-B), in1=pracc, op0=ALU.mult, op1=ALU.add)

            # ---------------- writeback ----------------
            nc.sync.dma_start(out=acc_out.ap(), in_=acc)

    nc.compile()
    meta = dict(T=T, G=G, NCOL=NCOL, COL_NEG=COL_NEG, COL_SELF=COL_SELF,
                COL_PAIR=COL_PAIR, COL_POS=COL_POS, COL_NCO=COL_NCO,
                COL_FOC=COL_FOC, COL_LS=COL_LS)
    _BUILD_CACHE[key] = (nc, meta)
    return nc, meta


def _host_prep(pred, target, features):
    """Build the 8 per-core input maps."""
    pred = np.asarray(pred, dtype=np.float32)
    labels = np.asarray(target).astype(np.int64)
    features = np.asarray(features, dtype=np.float32)

    feat_bf = features.astype(ml_dtypes.bfloat16)
    featT_bf = np.ascontiguousarray(feat_bf.T)             # [D, B]
    lab_i16 = labels.astype(np.int16)

    # same-label non-self pairs grouped by anchor
    order = np.argsort(labels, kind="stable")
    sorted_lab = labels[order]
    starts = np.flatnonzero(np.r_[True, sorted_lab[1:] != sorted_lab[:-1]])
    ends = np.r_[starts[1:], len(sorted_lab)]
    partners = [[] for _ in range(B)]
    k_real = 0
    for s, e in zip(starts, ends):
        if e - s < 2:
            continue
        members = order[s:e]
        for a in members:
            for p in members:
                if p != a:
                    partners[a].append(p)
                    k_real += 1

    # anchor-slot packing: each slot = (anchor, up to J partners)
    slots = [[[] for _ in range(RT)] for _ in range(N_CORES)]
    for a in range(B):
        ps = partners[a]
        if not ps:
            continue
        c, m = a // R, (a % R) // 128
        for o in range(0, len(ps), J):
            slots[c][m].append((a, ps[o:o + J]))
    T_m = [max(1, max(math.ceil(len(slots[c][m]) / 128)
                      for c in range(N_CORES))) for m in range(RT)]
    T0, T1 = T_m
    T = T0 + T1
    G = T * J

    e1T = np.zeros((D,), ml_dtypes.bfloat16)
    e1T[0] = 1.0

    in_maps = []
    for c in range(N_CORES):
        rows = slice(c * R, (c + 1) * R)
        ftl2 = featT_bf[:, rows].reshape(KT, 128, R)
        ftl2 = np.ascontiguousarray(ftl2.transpose(1, 0, 2).reshape(
            128, KT * R))
        fl = feat_bf[rows].reshape(RT, 128, D)
        fl2 = np.ascontiguousarray(fl.transpose(1, 0, 2).reshape(
            128, RT * D))
        pr = pred[rows].astype(ml_dtypes.bfloat16).reshape(RT, 128, C)
        pr2 = np.ascontiguousarray(pr.transpose(1, 0, 2).reshape(
            128, RT * C))
        lab_loc2 = np.ascontiguousarray(
            lab_i16[rows].reshape(RT, 128).T.astype(np.float32))

        fiT = np.empty((D, T * 128), ml_dtypes.bfloat16)
        fpT = np.empty((D, G * 128), ml_dtypes.bfloat16)
        fiT[:] = e1T[:, None]
        fpT[:] = e1T[:, None]
        rowidx = np.zeros((T * 128,), np.int16)
        valid = np.zeros((128, G), np.float32)
        toff = 0
        for m in range(RT):
            sl = slots[c][m]
            for si, (a, ps) in enumerate(sl):
                t = toff + si // 128
                p = si % 128
                fiT[:, t * 128 + p] = featT_bf[:, a]
                rowidx[t * 128 + p] = a % 128
                for j, pp in enumerate(ps):
                    fpT[:, (t * J + j) * 128 + p] = featT_bf[:, pp]
                    valid[p, t * J + j] = 1.0
            toff += T_m[m]
        pfiT = np.ascontiguousarray(
            fiT.reshape(KT, 128, T * 128).transpose(1, 0, 2).reshape(
                128, KT * T * 128))
        pfpT = np.ascontiguousarray(
            fpT.reshape(KT, 128, G * 128).transpose(1, 0, 2).reshape(
                128, KT * G * 128))
        idx16 = rowidx.reshape(T, 8, 16).transpose(2, 0, 1).reshape(16, -1)
        pidx = np.ascontiguousarray(np.tile(idx16, (8, 1)))

        in_maps.append({
            "featT": featT_bf,
            "ftl2": ftl2,
            "featl2": fl2,
            "pred2": pr2,
            "lab_row": np.ascontiguousarray(lab_i16[None, :]),
            "lab_loc2": lab_loc2,
            "pfiT": pfiT,
            "pfpT": pfpT,
            "pidx": pidx,
            "pval": valid,
        })
    return in_maps, T0, T1, k_real


def _combine(results, meta, k_real):
    """Host-side scalar all-reduce + final loss combination."""
    accs = np.stack([r["acc_out"] for r in results]).astype(np.float64)
    tot = accs.sum(axis=(0, 1))                 # [NCOL]

    neg_dense = -(tot[meta["COL_NEG"]] + tot[meta["COL_NEG"] + 1]
                  - C_MARGIN * B * B)
    self_trip = -(tot[meta["COL_SELF"]] + tot[meta["COL_SELF"] + 1]
                  - T_MARGIN * B * B)
    pair_trip = -tot[meta["COL_PAIR"]:meta["COL_PAIR"] + meta["G"]].sum()
    pair_pos = tot[meta["COL_POS"]]
    nco = tot[meta["COL_NCO"]]
    focal_sum = tot[meta["COL_FOC"]] + tot[meta["COL_FOC"] + 1]
    ls_sum = tot[meta["COL_LS"]] + tot[meta["COL_LS"] + 1]

    k_tot = k_real + B
    pos_self = B * (-np.log(np.exp(1.0 / TEMPERATURE) + 1e-8))
    pos_zero = (B * B - k_tot) * (-np.log1p(1e-8))
    pos_sum = pair_pos + pos_self + pos_zero
    neg_sum = neg_dense + nco + 0.5 * B

    lc = (pos_sum + neg_sum) / (B * B)
    lt = (self_trip + pair_trip) / (B + 1e-8)
    lf = ALPHA * focal_sum / B
    ls = ls_sum / B
    total = (W_CONTRASTIVE * lc + W_TRIPLET * lt
             + W_FOCAL * lf + W_LABEL_SMOOTH * ls)
    return np.array([lc, lt, lf, ls, total], dtype=np.float32)


def kernel(pred, target, features):
    in_maps, T0, T1, k_real = _host_prep(pred, target, features)
    nc, meta = _build(T0, T1)
    res = run_bass_kernel_spmd(nc, in_maps, core_ids=list(range(N_CORES)))
    return _combine(res.results, meta, k_real)


if __name__ == "__main__":
    import reference

    inputs = reference.setup_inputs()
    expected = np.asarray(reference.reference(**inputs))
    actual = kernel(**{k: np.asarray(v) for k, v in inputs.items()})
    err = np.abs(actual - expected) / np.maximum(np.abs(expected), 1e-12)
    print("expected:", expected)
    print("actual:  ", actual)
    print("rel err: ", err)



# revision 5
# speedup vs baseline: 1.5188x; 1.5188x over previous
"""Trainium2 Bass kernel v4 for nn_EnhancedLossModule.

Per-core plan (8 cores, 256 rows each, SPMD-uniform program):
  - Host precomputes fp8(e4m3) features Q, normalized Qn, exact row
    norms r of Q, and all same-label pair dot products (thresholds for
    the triplet pair reductions + contrastive pair corrections).
  - d2 = r_i + r_j - 2*Q_i.Q_j via fp8 DoubleRow matmuls (0.5 cyc/row)
    + one fp16 rank-1 matmul (ones x r_row) + a tiny one-hot matmul
    that adds 4096 to each row's own column (diag mask, NaN-safe sqrt).
    Columns are rotated by c*R per core so the diag block is always in
    chunk 0 -> the program is identical on every core.
  - ACT does sqrt(psum + r_i) -> dpt fp16; the ONLY other ACT work is
    the focal exp pass, so exactly 2 activation-table loads occur.
  - Triplet reductions: sum_n min(d', x) via DVE tensor_scalar
    min+accum passes: threshold 1.0 (self term), S0/S1 per-anchor
    threshold columns (rows sorted by partner count, heavy half in
    tile m0), plus one packed tile (PE one-hot row-select matmul of
    leftover anchors) reduced straight from PSUM.
  - sim = Qn_i.Qn_j via fp8 DoubleRow matmuls; sum_n min(sim, 0.5)
    reduced straight from PSUM (DVE).
  - Focal/label-smoothing: HW computes per-row sum(exp(pred_bf16));
    host does the O(B) log/pow tail, pred[i,target_i] and row sums.
  - Host combines everything exactly (corrections for diag/self and
    same-label columns are computed from the pair dot products).
"""

import ml_dtypes
import numpy as np

import concourse.bacc as bacc
import concourse.bass as bass
import concourse.tile as tile
from concourse import mybir
from concourse.bass_utils import run_bass_kernel_spmd

B, C, D = 2048, 1000, 512
N_CORES = 8
R = B // N_CORES            # 256 rows per core
RT = R // 128               # 2 row tiles
KT = D // 128               # 4 contraction tiles (2 DoubleRow pairs)

TEMPERATURE = 0.07
C_MARGIN = 0.5
T_MARGIN = 1.0
GAMMA = 2.0
ALPHA = 0.25
SMOOTHING = 0.1
W_CONTRASTIVE = 0.1
W_TRIPLET = 0.1
W_FOCAL = 0.4
W_LABEL_SMOOTH = 0.4

DIAG = 4096.0               # added to d2 of each row's own column
OFF = SMOOTHING / (C - 1)

F32 = mybir.dt.float32
BF16 = mybir.dt.bfloat16
FP16 = mybir.dt.float16
F8E4 = mybir.dt.float8e4
ALU = mybir.AluOpType
AF = mybir.ActivationFunctionType
E4M3 = ml_dtypes.float8_e4m3fn

_BUILD_CACHE: dict = {}


def _ap3(t, off, d1, n1, d2, n2):
    """3-dim AP view of a 2-D tile: [[pstride,128],[d1,n1],[d2,n2]]."""
    a = t[:, :]
    return bass.AP(tensor=a.tensor, offset=a.offset + off,
                   ap=[[a.ap[0][0], 128], [d1, n1], [d2, n2]])


def _build(S0: int, S1: int):
    key = (S0, S1)
    if key in _BUILD_CACHE:
        return _BUILD_CACHE[key]
    SD = S0 + S1

    # accumulator columns
    C_SELF = 0                  # RT: sum min(d', 1) per row tile
    C_DIR = C_SELF + RT         # SD: per-anchor pair sums
    C_PK = C_DIR + SD           # 1: packed pair sums
    C_SIM = C_PK + 1            # RT: sum min(sim, 0.5)
    C_SE = C_SIM + RT           # RT: sum exp(pred)
    NCOL = C_SE + RT

    nc = bacc.Bacc("TRN2", target_bir_lowering=False, debug=False,
                   num_devices=N_CORES)

    ft8 = nc.dram_tensor("ft8", [128, KT * B], F8E4, kind="ExternalInput")
    fn8 = nc.dram_tensor("fn8", [128, KT * B], F8E4, kind="ExternalInput")
    fl8 = nc.dram_tensor("fl8", [128, KT * R], F8E4, kind="ExternalInput")
    fnl8 = nc.dram_tensor("fnl8", [128, KT * R], F8E4,
                          kind="ExternalInput")
    rrow = nc.dram_tensor("rrow", [1, B], FP16, kind="ExternalInput")
    pred2 = nc.dram_tensor("pred2", [128, RT * C], BF16,
                           kind="ExternalInput")
    # aux f32: [rloc RT][colx RT][xdir SD][xpk 1]
    NAUX = RT + RT + SD + 1
    aux = nc.dram_tensor("aux", [128, NAUX], F32, kind="ExternalInput")
    selp = nc.dram_tensor("selp", [128, 128], FP16, kind="ExternalInput")
    acc_out = nc.dram_tensor("acc_out", [128, NCOL], F32,
                             kind="ExternalOutput")
    A_RLOC, A_COLX, A_XDIR, A_XPK = 0, RT, 2 * RT, 2 * RT + SD

    with tile.TileContext(nc) as tc:
        with (
            tc.tile_pool(name="persist", bufs=1) as persist,
            tc.tile_pool(name="scratch", bufs=2) as scratch,
            tc.tile_pool(name="gpsum", bufs=2, space="PSUM") as gpsum,
        ):
            # ---------------- inputs ----------------
            ft = persist.tile([128, KT * B], F8E4)
            nc.sync.dma_start(out=ft[:, :2 * B], in_=ft8.ap()[:, :2 * B])
            nc.sync.dma_start(out=ft[:, 2 * B:], in_=ft8.ap()[:, 2 * B:])
            fl = persist.tile([128, KT * R], F8E4)
            nc.sync.dma_start(out=fl, in_=fl8.ap())
            rro = persist.tile([1, B], FP16)
            nc.sync.dma_start(out=rro, in_=rrow.ap())
            auxt = persist.tile([128, NAUX], F32)
            nc.gpsimd.dma_start(out=auxt, in_=aux.ap())
            fn = persist.tile([128, KT * B], F8E4)
            nc.sync.dma_start(out=fn[:, :2 * B], in_=fn8.ap()[:, :2 * B])
            nc.sync.dma_start(out=fn[:, 2 * B:], in_=fn8.ap()[:, 2 * B:])
            fnl = persist.tile([128, KT * R], F8E4)
            nc.sync.dma_start(out=fnl, in_=fnl8.ap())
            sel = persist.tile([128, 128], FP16)
            nc.gpsimd.dma_start(out=sel, in_=selp.ap())
            pr2 = persist.tile([128, RT * C], BF16)
            nc.gpsimd.dma_start(out=pr2, in_=pred2.ap())

            # ---------------- constants ----------------
            iota256 = persist.tile([128, 256], F32)
            nc.gpsimd.iota(iota256, pattern=[[1, 256]], base=0,
                           channel_multiplier=0,
                           allow_small_or_imprecise_dtypes=True)
            iota128 = persist.tile([128, 128], F32)
            nc.gpsimd.iota(iota128, pattern=[[1, 128]], base=0,
                           channel_multiplier=0,
                           allow_small_or_imprecise_dtypes=True)
            pid = persist.tile([128, 1], F32)
            nc.gpsimd.iota(pid, pattern=[[0, 1]], base=0,
                           channel_multiplier=1,
                           allow_small_or_imprecise_dtypes=True)
            ident = persist.tile([128, 128], FP16)
            nc.vector.tensor_scalar(out=ident, in0=iota128, scalar1=pid,
                                    scalar2=None, op0=ALU.is_equal)
            ones1 = persist.tile([1, 128], FP16)
            nc.vector.memset(ones1, 1.0)
            # P_m: 4096 at each sorted row's own (rotated) column
            pm = persist.tile([128, RT * 256], FP16)
            for m in range(RT):
                nc.vector.tensor_scalar(
                    out=pm[:, m * 256:(m + 1) * 256], in0=iota256,
                    scalar1=auxt[:, A_COLX + m:A_COLX + m + 1],
                    scalar2=DIAG, op0=ALU.is_equal, op1=ALU.mult)

            junk_v = persist.tile([128, B], FP16)
            junk_a = persist.tile([128, C], BF16)
            acc = persist.tile([128, NCOL], F32)
            nc.vector.memset(acc, 0.0)

            dpt = [persist.tile([128, B], FP16, name=f"dpt{m}")
                   for m in range(RT)]

            # ---------------- d2 matmuls + sqrt + reductions ----------
            d2ps = []
            for m in range(RT):
                ps = gpsum.tile([128, B], F32, tag="big", name=f"d2ps{m}")
                d2ps.append(ps)
                for ch in range(4):
                    o = ch * 512
                    for kp in range(KT // 2):
                        nc.tensor.matmul(
                            ps[:, o:o + 512],
                            _ap3(fl, 2 * kp * R + m * 128, R, 2, 1, 128),
                            _ap3(ft, 2 * kp * B + o, B, 2, 1, 512),
                            start=(kp == 0), stop=False,
                            perf_mode=mybir.MatmulPerfMode.DoubleRow,
                        )
                    if ch == 0:
                        # diag mask: += ident^T @ P_m (first 256 cols)
                        nc.tensor.matmul(
                            ps[:, 0:256], ident[:, :],
                            pm[:, m * 256:(m + 1) * 256],
                            start=False, stop=False,
                            skip_group_check=True,
                        )
                    # += ones^T x rrow (fp16 rank-1)
                    nc.tensor.matmul(
                        ps[:, o:o + 512], ones1[0:1, :],
                        rro[0:1, o:o + 512],
                        start=False, stop=True, skip_group_check=True,
                    )

            for m in range(RT):
                # dpt = sqrt(psum + r_i)
                nc.scalar.activation(
                    out=dpt[m], in_=d2ps[m], func=AF.Sqrt,
                    bias=auxt[:, A_RLOC + m:A_RLOC + m + 1])
                # self: sum min(d', 1)
                nc.vector.tensor_scalar(
                    out=junk_v, in0=dpt[m], scalar1=float(T_MARGIN),
                    scalar2=None, op0=ALU.min, op1=ALU.add,
                    accum_out=acc[:, C_SELF + m:C_SELF + m + 1])
                # direct pair passes
                ns = S0 if m == 0 else S1
                off = 0 if m == 0 else S0
                for s in range(ns):
                    nc.vector.tensor_scalar(
                        out=junk_v, in0=dpt[m],
                        scalar1=auxt[:, A_XDIR + off + s:
                                     A_XDIR + off + s + 1],
                        scalar2=None, op0=ALU.min, op1=ALU.add,
                        accum_out=acc[:, C_DIR + off + s:
                                      C_DIR + off + s + 1])

            # ---------------- sim matmuls + reductions ----------------
            for m in range(RT):
                ps = gpsum.tile([128, B], F32, tag="big", name=f"simps{m}")
                for ch in range(4):
                    o = ch * 512
                    for kp in range(KT // 2):
                        nc.tensor.matmul(
                            ps[:, o:o + 512],
                            _ap3(fnl, 2 * kp * R + m * 128, R, 2, 1, 128),
                            _ap3(fn, 2 * kp * B + o, B, 2, 1, 512),
                            start=(kp == 0), stop=(kp == KT // 2 - 1),
                            perf_mode=mybir.MatmulPerfMode.DoubleRow,
                        )
                nc.vector.tensor_scalar(
                    out=junk_v, in0=ps, scalar1=float(C_MARGIN),
                    scalar2=None, op0=ALU.min, op1=ALU.add,
                    accum_out=acc[:, C_SIM + m:C_SIM + m + 1])

            # ---------------- packed leftover pairs ----------------
            pspk = gpsum.tile([128, B], F32, tag="big", name="pspk")
            for m in range(RT):
                for ch in range(4):
                    o = ch * 512
                    nc.tensor.matmul(
                        pspk[m * 64:(m + 1) * 64, o:o + 512],
                        sel[:, m * 64:(m + 1) * 64],
                        dpt[m][:, o:o + 512],
                        start=True, stop=True,
                    )
            nc.vector.tensor_scalar(
                out=junk_v, in0=pspk,
                scalar1=auxt[:, A_XPK:A_XPK + 1],
                scalar2=None, op0=ALU.min, op1=ALU.add,
                accum_out=acc[:, C_PK:C_PK + 1])

            # ---------------- focal exp (single table switch) ---------
            with tc.tile_wait_until(0.012):
                for m in range(RT):
                    nc.scalar.activation(
                        out=junk_a, in_=pr2[:, m * C:(m + 1) * C],
                        func=AF.Exp,
                        accum_out=acc[:, C_SE + m:C_SE + m + 1])

            nc.sync.dma_start(out=acc_out.ap(), in_=acc)

    nc.compile()
    meta = dict(S0=S0, S1=S1, NCOL=NCOL, C_SELF=C_SELF, C_DIR=C_DIR,
                C_PK=C_PK, C_SIM=C_SIM, C_SE=C_SE)
    _BUILD_CACHE[key] = (nc, meta)
    return nc, meta


def _host_prep(pred, target, features):
    pred = np.asarray(pred, dtype=np.float64)
    lab = np.asarray(target).astype(np.int64)
    f = np.asarray(features, dtype=np.float64)

    Q = f.astype(np.float32).astype(E4M3)
    Qf = Q.astype(np.float64)
    r = np.einsum("ij,ij->i", Qf, Qf)                  # exact fp8 row norms
    rr16 = r.astype(np.float16)                        # column add values
    rr16f = rr16.astype(np.float64)

    nrm = np.linalg.norm(f, axis=1)
    n = f / nrm[:, None]
    Qn = n.astype(np.float32).astype(E4M3)
    Qnf = Qn.astype(np.float64)

    # same-label partner lists
    order = np.argsort(lab, kind="stable")
    sl = lab[order]
    starts = np.flatnonzero(np.r_[True, sl[1:] != sl[:-1]])
    ends = np.r_[starts[1:], len(sl)]
    partners = [[] for _ in range(B)]
    for s, e in zip(starts, ends):
        if e - s < 2:
            continue
        mem = order[s:e]
        for a in mem:
            for p in mem:
                if p != a:
                    partners[a].append(int(p))
    pcnt = np.array([len(p) for p in partners])
    NP = int(pcnt.sum())

    # in-core sort by partner count (heavy anchors into tile m0)
    corder = np.empty((N_CORES, R), np.int64)
    for c in range(N_CORES):
        seg = np.arange(c * R, (c + 1) * R)
        corder[c] = seg[np.argsort(-pcnt[seg], kind="stable")]

    # direct slot counts (uniform across cores)
    S0 = min(3, int(max(pcnt[corder[c][:128]].max() for c in range(N_CORES))))
    S1 = min(3, int(max(pcnt[corder[c][128:]].max() for c in range(N_CORES))))
    S0 = max(S0, 1)
    S1 = max(S1, 1)

    # leftovers -> packed slots (64 per row tile)
    while True:
        ok = True
        for c in range(N_CORES):
            for m in range(RT):
                Sm = S0 if m == 0 else S1
                left = sum(max(0, pcnt[g] - Sm)
                           for g in corder[c][m * 128:(m + 1) * 128])
                if left > 64:
                    ok = False
        if ok:
            break
        if S0 < 7:
            S0 += 1
        elif S1 < 7:
            S1 += 1
        else:
            raise RuntimeError("packed overflow")

    # pair values (Q-space distances; original-space + Qn-space sims)
    #   d2q[a][k] matches HW: fp32(r_a) + fp16(r_p) - 2 Q_a.Q_p
    dq = [None] * B
    sim_true = [None] * B
    sim_q = [None] * B
    for a in range(B):
        ps = partners[a]
        if not ps:
            continue
        P = np.array(ps)
        g = Qf[P] @ Qf[a]
        d2 = r[a] + rr16f[P] - 2.0 * g
        dq[a] = np.sqrt(np.maximum(d2, 0.0))
        sim_true[a] = (f[P] @ f[a]) / (nrm[a] * nrm[P])
        sim_q[a] = Qnf[P] @ Qnf[a]

    SD = S0 + S1
    in_maps = []
    xdir_all = np.zeros((N_CORES, 128, SD), np.float64)
    xpk_all = np.zeros((N_CORES, 128), np.float64)
    pk_map = [[] for _ in range(N_CORES)]   # (slot, anchor, partner_idx)
    dir_map = [[] for _ in range(N_CORES)]  # (p, m, s, anchor, partner_idx)

    ftT = np.ascontiguousarray(Q.T)          # [D, B]
    fnT = np.ascontiguousarray(Qn.T)

    for c in range(N_CORES):
        rot = np.roll(np.arange(B), -c * R)  # rotated column order
        ft8 = np.empty((128, KT * B), E4M3)
        fn8 = np.empty((128, KT * B), E4M3)
        for k in range(KT):
            ft8[:, k * B:(k + 1) * B] = ftT[k * 128:(k + 1) * 128, rot]
            fn8[:, k * B:(k + 1) * B] = fnT[k * 128:(k + 1) * 128, rot]
        rows = corder[c]
        fl8 = np.empty((128, KT * R), E4M3)
        fnl8 = np.empty((128, KT * R), E4M3)
        m2q = (-2.0 * Qf[rows]).astype(E4M3)      # exact in e4m3
        for k in range(KT):
            fl8[:, k * R:(k + 1) * R] = m2q[:, k * 128:(k + 1) * 128].T
            fnl8[:, k * R:(k + 1) * R] = \
                fnT[k * 128:(k + 1) * 128][:, rows]
        rrow_h = rr16[rot][None, :]

        auxh = np.zeros((128, 2 * RT + SD + 1), np.float32)
        selh = np.zeros((128, 128), np.float16)
        nslot = [0, 0]
        for m in range(RT):
            Sm = S0 if m == 0 else S1
            soff = 0 if m == 0 else S0
            for p in range(128):
                g = int(rows[m * 128 + p])
                auxh[p, m] = r[g]                     # rloc (fp32 bias)
                auxh[p, RT + m] = (g - c * R) % B     # colx (rotated)
                ps = partners[g]
                for s in range(min(len(ps), Sm)):
                    x = dq[g][s] + T_MARGIN
                    auxh[p, 2 * RT + soff + s] = x
                    xdir_all[c, p, soff + s] = x
                    dir_map[c].append((p, m, s, g, s))
                for s in range(Sm, len(ps)):
                    slot = m * 64 + nslot[m]
                    nslot[m] += 1
                    selh[p, slot] = 1.0
                    x = dq[g][s] + T_MARGIN
                    auxh[slot, 2 * RT + SD] = x
                    xpk_all[c, slot] = x
                    pk_map[c].append((slot, g, s))
        assert max(nslot) <= 64

        pr = np.asarray(pred, np.float32)[c * R:(c + 1) * R]
        pr = pr.astype(ml_dtypes.bfloat16).reshape(RT, 128, C)
        pr2 = np.ascontiguousarray(pr.transpose(1, 0, 2).reshape(
            128, RT * C))

        in_maps.append({
            "ft8": ft8, "fn8": fn8, "fl8": fl8, "fnl8": fnl8,
            "rrow": rrow_h, "pred2": pr2, "aux": auxh, "selp": selh,
        })

    prep = dict(S0=S0, S1=S1, NP=NP, partners=partners, pcnt=pcnt,
                corder=corder, dq=dq, sim_true=sim_true, sim_q=sim_q,
                dir_map=dir_map, pk_map=pk_map, xdir=xdir_all,
                xpk=xpk_all, pred=pred, lab=lab)
    return in_maps, prep


def _combine(results, meta, prep):
    accs = np.stack([r["acc_out"] for r in results]).astype(np.float64)
    S0, S1 = prep["S0"], prep["S1"]
    partners, pcnt = prep["partners"], prep["pcnt"]
    dq, sim_true, sim_q = prep["dq"], prep["sim_true"], prep["sim_q"]
    NP = prep["NP"]
    Bf = float(B)

    # ---- contrastive ----
    pos_pair = sum(
        -np.log(np.exp(np.asarray(sim_true[a]) / TEMPERATURE) + 1e-8).sum()
        for a in range(B) if sim_true[a] is not None)
    pos_sum = (pos_pair
               + B * (-np.log(np.exp(1.0 / TEMPERATURE) + 1e-8))
               + (Bf * Bf - B - NP) * (-np.log1p(1e-8)))

    M = accs[:, :, meta["C_SIM"]:meta["C_SIM"] + RT].sum()
    pair_min = sum(np.minimum(np.asarray(sim_q[a]), C_MARGIN).sum()
                   for a in range(B) if sim_q[a] is not None)
    sum_min_diff = M - C_MARGIN * B - pair_min
    neg_sum = 0.5 * Bf * Bf - sum_min_diff
    lc = (pos_sum + neg_sum) / (Bf * Bf)

    # ---- triplet self term ----
    selfsum = accs[:, :, meta["C_SELF"]:meta["C_SELF"] + RT].sum()
    mp = sum(np.minimum(dq[a], T_MARGIN).sum()
             for a in range(B) if dq[a] is not None)
    n_diff_sum = Bf * Bf - (B + NP)
    self_part = n_diff_sum * T_MARGIN - (selfsum - B * T_MARGIN - mp)

    # ---- triplet pair term ----
    pair_part = 0.0
    for c in range(N_CORES):
        acc_c = accs[c]
        for (p, m, s, a, k) in prep["dir_map"][c]:
            x = prep["xdir"][c, p, (0 if m == 0 else S0) + s]
            S_ap = acc_c[p, meta["C_DIR"] + (0 if m == 0 else S0) + s]
            corr = np.maximum(x - dq[a], 0.0).sum()
            pair_part += Bf * x - S_ap - corr
        for (slot, a, k) in prep["pk_map"][c]:
            x = prep["xpk"][c, slot]
            S_ap = acc_c[slot, meta["C_PK"]]
            corr = np.maximum(x - dq[a], 0.0).sum()
            pair_part += Bf * x - S_ap - corr
    lt = (self_part + pair_part) / (Bf + 1e-8)

    # ---- focal + label smoothing (host tail) ----
    pred, lab = prep["pred"], prep["lab"]
    se = np.empty(B)
    for c in range(N_CORES):
        for m in range(RT):
            se[c * R + m * 128:c * R + (m + 1) * 128] = \
                accs[c, :, meta["C_SE"] + m]
    lse = np.log(se)
    ptgt = pred[np.arange(B), lab]
    spred = pred.sum(axis=1)
    ce = lse - ptgt
    pt = np.exp(-ce)
    lf = (ALPHA * (1.0 - pt) ** GAMMA * ce).mean()
    ls = (-(OFF * (spred - C * lse)
            + (1.0 - SMOOTHING - OFF) * (ptgt - lse))).mean()

    total = (W_CONTRASTIVE * lc + W_TRIPLET * lt
             + W_FOCAL * lf + W_LABEL_SMOOTH * ls)
    return np.array([lc, lt, lf, ls, total], dtype=np.float32)


def kernel(pred, target, features):
    in_maps, prep = _host_prep(pred, target, features)
    nc, meta = _build(prep["S0"], prep["S1"])
    res = run_bass_kernel_spmd(nc, in_maps, core_ids=list(range(N_CORES)))
    return _combine(res.results, meta, prep)


if __name__ == "__main__":
    import reference

    inputs = reference.setup_inputs()
    expected = np.asarray(reference.reference(**inputs))
    actual = kernel(**{k: np.asarray(v) for k, v in inputs.items()})
    err = np.abs(actual - expected) / np.maximum(np.abs(expected), 1e-12)
    print("expected:", expected)
    print("actual:  ", actual)
    print("rel err: ", err)


# revision 6
# speedup vs baseline: 1.9184x; 1.2631x over previous
"""Trainium2 Bass kernel v5 for nn_EnhancedLossModule.

Per-core plan (8 cores, 256 rows each, SPMD-uniform program):
  - Host precomputes fp8(e4m3) features Q, normalized Qn, exact row
    norms r of Q, and all same-label pair dot products (thresholds for
    the triplet pair reductions + contrastive pair corrections).
  - d2 = r_i + r_j - 2*Q_i.Q_j via fp8 DoubleRow matmuls (0.5 cyc/row)
    + one fp16 rank-1 matmul (ones x r_row) + a tiny one-hot matmul
    that adds 4096 to each row's own column (diag mask, NaN-safe sqrt).
    Columns are rotated by c*R per core so the diag block is always in
    the first 256 columns -> the program is identical on every core.
  - PSUM is used as [128, 1024] half-tiles (2 banks each, 4 in flight)
    so PE streams without bank stalls; warm-up matmuls ramp the PE
    p-state before the real work arrives.
  - ACT does sqrt(psum + r_i) -> dpt fp16 and the focal exp pass ->
    exactly 2 activation-table loads.
  - Triplet reductions: sum_n min(d', x) via DVE tensor_scalar
    min+accum passes: threshold 1.0 (self term) plus S0/S1 per-anchor
    threshold columns (rows sorted by partner count so the heavy
    anchors share tile m0).
  - sim = Qn_i.Qn_j via fp8 DoubleRow matmuls; sum_n min(sim, 0.5)
    reduced straight from PSUM halves (DVE).
  - Focal/label-smoothing: HW computes per-row sum(exp(pred_bf16));
    host does the O(B) log/pow tail.
"""

import os

import ml_dtypes
import numpy as np

import concourse.bacc as bacc
import concourse.bass as bass
import concourse.tile as tile
from concourse import mybir
from concourse.bass_utils import run_bass_kernel_spmd

B, C, D = 2048, 1000, 512
N_CORES = 8
R = B // N_CORES            # 256 rows per core
RT = R // 128               # 2 row tiles
KT = D // 128               # 4 contraction tiles (2 DoubleRow pairs)
HB = B // 2                 # psum half-tile width

TEMPERATURE = 0.07
C_MARGIN = 0.5
T_MARGIN = 1.0
GAMMA = 2.0
ALPHA = 0.25
SMOOTHING = 0.1
W_CONTRASTIVE = 0.1
W_TRIPLET = 0.1
W_FOCAL = 0.4
W_LABEL_SMOOTH = 0.4

DIAG = 4096.0               # added to d2 of each row's own column
OFF = SMOOTHING / (C - 1)
WARMN = int(os.environ.get("WARMN", "22"))
EXPWAIT = float(os.environ.get("EXPWAIT_MS", "0.0085"))

F32 = mybir.dt.float32
BF16 = mybir.dt.bfloat16
FP16 = mybir.dt.float16
F8E4 = mybir.dt.float8e4
ALU = mybir.AluOpType
AF = mybir.ActivationFunctionType
E4M3 = ml_dtypes.float8_e4m3fn

_BUILD_CACHE: dict = {}


def _ap3(t, off, d1, n1, d2, n2):
    """3-dim AP view of a 2-D tile: [[pstride,128],[d1,n1],[d2,n2]]."""
    a = t[:, :]
    return bass.AP(tensor=a.tensor, offset=a.offset + off,
                   ap=[[a.ap[0][0], 128], [d1, n1], [d2, n2]])


def _build(S0: int, S1: int):
    key = (S0, S1)
    if key in _BUILD_CACHE:
        return _BUILD_CACHE[key]
    SD = S0 + S1

    # accumulator columns
    C_SELF = 0                  # RT: sum min(d', 1) per row tile
    C_DIR = C_SELF + RT         # SD: per-anchor pair sums
    C_SIM = C_DIR + SD          # RT: sum min(sim, 0.5)
    C_SE = C_SIM + RT           # RT: sum exp(pred)
    NCOL = C_SE + RT

    nc = bacc.Bacc("TRN2", target_bir_lowering=False, debug=False,
                   num_devices=N_CORES)

    ft8 = nc.dram_tensor("ft8", [128, KT * B], F8E4, kind="ExternalInput")
    fn8 = nc.dram_tensor("fn8", [128, KT * B], F8E4, kind="ExternalInput")
    fl8 = nc.dram_tensor("fl8", [128, KT * R], F8E4, kind="ExternalInput")
    fnl8 = nc.dram_tensor("fnl8", [128, KT * R], F8E4,
                          kind="ExternalInput")
    rrow = nc.dram_tensor("rrow", [1, B], FP16, kind="ExternalInput")
    pred2 = nc.dram_tensor("pred2", [128, RT * C], BF16,
                           kind="ExternalInput")
    # aux f32: [rloc RT][colx RT][xdir SD]
    NAUX = 2 * RT + SD
    aux = nc.dram_tensor("aux", [128, NAUX], F32, kind="ExternalInput")
    acc_out = nc.dram_tensor("acc_out", [128, NCOL], F32,
                             kind="ExternalOutput")
    A_RLOC, A_COLX, A_XDIR = 0, RT, 2 * RT

    with tile.TileContext(nc) as tc:
        with (
            tc.tile_pool(name="persist", bufs=1) as persist,
            tc.tile_pool(name="gpsum", bufs=4, space="PSUM") as gpsum,
        ):
            # ---------------- inputs ----------------
            # critical path first: fl, ft halves; fn later; pred last.
            fl = persist.tile([128, KT * R], F8E4)
            nc.sync.dma_start(out=fl, in_=fl8.ap())
            ft = persist.tile([128, KT * B], F8E4)
            nc.sync.dma_start(out=ft[:, :2 * B], in_=ft8.ap()[:, :2 * B])
            nc.sync.dma_start(out=ft[:, 2 * B:], in_=ft8.ap()[:, 2 * B:])
            fnl = persist.tile([128, KT * R], F8E4)
            nc.sync.dma_start(out=fnl, in_=fnl8.ap())
            fn = persist.tile([128, KT * B], F8E4)
            nc.sync.dma_start(out=fn[:, :2 * B], in_=fn8.ap()[:, :2 * B])
            nc.sync.dma_start(out=fn[:, 2 * B:], in_=fn8.ap()[:, 2 * B:])
            pr2 = persist.tile([128, RT * C], BF16)
            nc.sync.dma_start(out=pr2, in_=pred2.ap())
            auxt = persist.tile([128, NAUX], F32)
            nc.scalar.dma_start(out=auxt, in_=aux.ap())
            rro = persist.tile([1, B], FP16)
            nc.scalar.dma_start(out=rro, in_=rrow.ap())

            # ---------------- constants ----------------
            iota256 = persist.tile([128, 256], F32)
            nc.gpsimd.iota(iota256, pattern=[[1, 256]], base=0,
                           channel_multiplier=0,
                           allow_small_or_imprecise_dtypes=True)
            pid = persist.tile([128, 1], F32)
            nc.gpsimd.iota(pid, pattern=[[0, 1]], base=0,
                           channel_multiplier=1,
                           allow_small_or_imprecise_dtypes=True)
            ident = persist.tile([128, 128], FP16)
            nc.vector.tensor_scalar(out=ident, in0=iota256[:, 0:128],
                                    scalar1=pid, scalar2=None,
                                    op0=ALU.is_equal)
            ones1 = persist.tile([1, 128], FP16)
            nc.vector.memset(ones1, 1.0)
            # P_m: 4096 at each sorted row's own (rotated) column
            pm = persist.tile([128, RT * 256], FP16)
            for m in range(RT):
                nc.vector.tensor_scalar(
                    out=pm[:, m * 256:(m + 1) * 256], in0=iota256,
                    scalar1=auxt[:, A_COLX + m:A_COLX + m + 1],
                    scalar2=DIAG, op0=ALU.is_equal, op1=ALU.mult)

            junk_v = persist.tile([128, B], FP16)
            junk_a = persist.tile([128, C], BF16)
            acc = persist.tile([128, NCOL], F32)
            nc.vector.memset(acc, 0.0)

            dpt = [persist.tile([128, B], FP16, name=f"dpt{m}")
                   for m in range(RT)]

            # ---------------- d2 matmuls (psum half-tiles) ------------
            def dr_group(ps, src, lsrc, m, h, with_r):
                """One [128,1024] half: 2 chunks x (2 DoubleRow + extras)."""
                for ch in range(2):
                    o = ch * 512
                    go = h * 1024 + o
                    for kp in range(KT // 2):
                        nc.tensor.matmul(
                            ps[:, o:o + 512],
                            _ap3(lsrc, 2 * kp * R + m * 128, R, 2, 1, 128),
                            _ap3(src, 2 * kp * B + go, B, 2, 1, 512),
                            start=(kp == 0),
                            stop=(not with_r and kp == KT // 2 - 1),
                            perf_mode=mybir.MatmulPerfMode.DoubleRow,
                        )
                    if with_r:
                        if h == 0 and ch == 0:
                            nc.tensor.matmul(
                                ps[:, 0:256], ident[:, :],
                                pm[:, m * 256:(m + 1) * 256],
                                start=False, stop=False,
                                skip_group_check=True,
                            )
                        nc.tensor.matmul(
                            ps[:, o:o + 512], ones1[0:1, :],
                            rro[0:1, go:go + 512],
                            start=False, stop=True, skip_group_check=True,
                        )

            # warm-up matmuls: ramp the PE p-state while DMAs land
            wps = gpsum.tile([128, HB], F32, tag="big", name="warm")
            for w in range(WARMN):
                nc.tensor.matmul(wps[:, 0:128], ident[:, :], ident[:, :],
                                 start=True, stop=True,
                                 skip_group_check=True)

            d2ps = [[None] * 2 for _ in range(RT)]
            for m in range(RT):
                for h in range(2):
                    ps = gpsum.tile([128, HB], F32, tag="big",
                                    name=f"d2ps{m}{h}")
                    d2ps[m][h] = ps
                    dr_group(ps, ft, fl, m, h, with_r=True)

            simps = [[None] * 2 for _ in range(RT)]
            for m in range(RT):
                for h in range(2):
                    ps = gpsum.tile([128, HB], F32, tag="big",
                                    name=f"simps{m}{h}")
                    simps[m][h] = ps
                    dr_group(ps, fn, fnl, m, h, with_r=False)

            # ---------------- sqrt + triplet reductions ---------------
            for m in range(RT):
                for h in range(2):
                    nc.scalar.activation(
                        out=dpt[m][:, h * 1024:(h + 1) * 1024],
                        in_=d2ps[m][h], func=AF.Sqrt,
                        bias=auxt[:, A_RLOC + m:A_RLOC + m + 1])
                # self: sum min(d', 1)
                nc.vector.tensor_scalar(
                    out=junk_v, in0=dpt[m], scalar1=float(T_MARGIN),
                    scalar2=None, op0=ALU.min, op1=ALU.add,
                    accum_out=acc[:, C_SELF + m:C_SELF + m + 1])
                ns = S0 if m == 0 else S1
                off = 0 if m == 0 else S0
                for s in range(ns):
                    nc.vector.tensor_scalar(
                        out=junk_v, in0=dpt[m],
                        scalar1=auxt[:, A_XDIR + off + s:
                                     A_XDIR + off + s + 1],
                        scalar2=None, op0=ALU.min, op1=ALU.add,
                        accum_out=acc[:, C_DIR + off + s:
                                      C_DIR + off + s + 1])

            # ---------------- sim reductions ---------------------------
            for m in range(RT):
                for h in range(2):
                    nc.vector.tensor_scalar(
                        out=junk_v[:, 0:HB], in0=simps[m][h],
                        scalar1=float(C_MARGIN), scalar2=None,
                        op0=ALU.min, op1=ALU.add,
                        accum_out=acc[:, C_SIM + m:C_SIM + m + 1])

            # ---------------- focal exp (single table switch) ---------
            with tc.tile_wait_until(EXPWAIT):
                for m in range(RT):
                    nc.scalar.activation(
                        out=junk_a, in_=pr2[:, m * C:(m + 1) * C],
                        func=AF.Exp,
                        accum_out=acc[:, C_SE + m:C_SE + m + 1])

            nc.sync.dma_start(out=acc_out.ap(), in_=acc)

    nc.compile()
    meta = dict(S0=S0, S1=S1, NCOL=NCOL, C_SELF=C_SELF, C_DIR=C_DIR,
                C_SIM=C_SIM, C_SE=C_SE)
    _BUILD_CACHE[key] = (nc, meta)
    return nc, meta


def _host_prep(pred, target, features):
    pred = np.asarray(pred, dtype=np.float64)
    lab = np.asarray(target).astype(np.int64)
    f = np.asarray(features, dtype=np.float64)

    Q = f.astype(np.float32).astype(E4M3)
    Qf = Q.astype(np.float64)
    r = np.einsum("ij,ij->i", Qf, Qf)                  # exact fp8 row norms
    rr16 = r.astype(np.float16)
    rr16f = rr16.astype(np.float64)

    nrm = np.linalg.norm(f, axis=1)
    n = f / nrm[:, None]
    Qn = n.astype(np.float32).astype(E4M3)
    Qnf = Qn.astype(np.float64)

    # same-label partner lists
    order = np.argsort(lab, kind="stable")
    sl = lab[order]
    starts = np.flatnonzero(np.r_[True, sl[1:] != sl[:-1]])
    ends = np.r_[starts[1:], len(sl)]
    partners = [[] for _ in range(B)]
    for s, e in zip(starts, ends):
        if e - s < 2:
            continue
        mem = order[s:e]
        for a in mem:
            for p in mem:
                if p != a:
                    partners[a].append(int(p))
    pcnt = np.array([len(p) for p in partners])
    NP = int(pcnt.sum())

    # in-core sort by partner count (heavy anchors into tile m0)
    corder = np.empty((N_CORES, R), np.int64)
    for c in range(N_CORES):
        seg = np.arange(c * R, (c + 1) * R)
        corder[c] = seg[np.argsort(-pcnt[seg], kind="stable")]

    S0 = max(1, int(max(pcnt[corder[c][:128]].max()
                        for c in range(N_CORES))))
    S1 = max(1, int(max(pcnt[corder[c][128:]].max()
                        for c in range(N_CORES))))

    # pair values (Q-space distances; original/Qn-space sims)
    dq = [None] * B
    sim_true = [None] * B
    sim_q = [None] * B
    for a in range(B):
        ps = partners[a]
        if not ps:
            continue
        P = np.array(ps)
        g = Qf[P] @ Qf[a]
        d2 = r[a] + rr16f[P] - 2.0 * g
        dq[a] = np.sqrt(np.maximum(d2, 0.0))
        sim_true[a] = (f[P] @ f[a]) / (nrm[a] * nrm[P])
        sim_q[a] = Qnf[P] @ Qnf[a]

    SD = S0 + S1
    in_maps = []
    xdir_all = np.zeros((N_CORES, 128, SD), np.float64)
    dir_map = [[] for _ in range(N_CORES)]  # (p, m, s, anchor)

    ftT = np.ascontiguousarray(Q.T)          # [D, B]
    fnT = np.ascontiguousarray(Qn.T)

    for c in range(N_CORES):
        rot = np.roll(np.arange(B), -c * R)  # rotated column order
        ft8 = np.empty((128, KT * B), E4M3)
        fn8 = np.empty((128, KT * B), E4M3)
        for k in range(KT):
            ft8[:, k * B:(k + 1) * B] = ftT[k * 128:(k + 1) * 128, rot]
            fn8[:, k * B:(k + 1) * B] = fnT[k * 128:(k + 1) * 128, rot]
        rows = corder[c]
        fl8 = np.empty((128, KT * R), E4M3)
        fnl8 = np.empty((128, KT * R), E4M3)
        m2q = (-2.0 * Qf[rows]).astype(E4M3)      # exact in e4m3
        for k in range(KT):
            fl8[:, k * R:(k + 1) * R] = m2q[:, k * 128:(k + 1) * 128].T
            fnl8[:, k * R:(k + 1) * R] = \
                fnT[k * 128:(k + 1) * 128][:, rows]
        rrow_h = rr16[rot][None, :]

        auxh = np.zeros((128, 2 * RT + SD), np.float32)
        for m in range(RT):
            Sm = S0 if m == 0 else S1
            soff = 0 if m == 0 else S0
            for p in range(128):
                g = int(rows[m * 128 + p])
                auxh[p, m] = r[g]                     # rloc (fp32 bias)
                auxh[p, RT + m] = (g - c * R) % B     # colx (rotated)
                ps = partners[g]
                for s in range(min(len(ps), Sm)):
                    x = dq[g][s] + T_MARGIN
                    auxh[p, 2 * RT + soff + s] = x
                    xdir_all[c, p, soff + s] = x
                    dir_map[c].append((p, m, s, g))

        pr = np.asarray(pred, np.float32)[c * R:(c + 1) * R]
        pr = pr.astype(ml_dtypes.bfloat16).reshape(RT, 128, C)
        pr2 = np.ascontiguousarray(pr.transpose(1, 0, 2).reshape(
            128, RT * C))

        in_maps.append({
            "ft8": ft8, "fn8": fn8, "fl8": fl8, "fnl8": fnl8,
            "rrow": rrow_h, "pred2": pr2, "aux": auxh,
        })

    prep = dict(S0=S0, S1=S1, NP=NP, partners=partners, pcnt=pcnt,
                corder=corder, dq=dq, sim_true=sim_true, sim_q=sim_q,
                dir_map=dir_map, xdir=xdir_all, pred=pred, lab=lab)
    return in_maps, prep


def _combine(results, meta, prep):
    accs = np.stack([r["acc_out"] for r in results]).astype(np.float64)
    S0 = prep["S0"]
    dq, sim_true, sim_q = prep["dq"], prep["sim_true"], prep["sim_q"]
    NP = prep["NP"]
    Bf = float(B)

    # ---- contrastive ----
    pos_pair = sum(
        -np.log(np.exp(np.asarray(sim_true[a]) / TEMPERATURE) + 1e-8).sum()
        for a in range(B) if sim_true[a] is not None)
    pos_sum = (pos_pair
               + B * (-np.log(np.exp(1.0 / TEMPERATURE) + 1e-8))
               + (Bf * Bf - B - NP) * (-np.log1p(1e-8)))

    M = accs[:, :, meta["C_SIM"]:meta["C_SIM"] + RT].sum()
    pair_min = sum(np.minimum(np.asarray(sim_q[a]), C_MARGIN).sum()
                   for a in range(B) if sim_q[a] is not None)
    sum_min_diff = M - C_MARGIN * B - pair_min
    neg_sum = 0.5 * Bf * Bf - sum_min_diff
    lc = (pos_sum + neg_sum) / (Bf * Bf)

    # ---- triplet self term ----
    selfsum = accs[:, :, meta["C_SELF"]:meta["C_SELF"] + RT].sum()
    mp = sum(np.minimum(dq[a], T_MARGIN).sum()
             for a in range(B) if dq[a] is not None)
    n_diff_sum = Bf * Bf - (B + NP)
    self_part = n_diff_sum * T_MARGIN - (selfsum - B * T_MARGIN - mp)

    # ---- triplet pair term ----
    pair_part = 0.0
    for c in range(N_CORES):
        acc_c = accs[c]
        for (p, m, s, a) in prep["dir_map"][c]:
            x = prep["xdir"][c, p, (0 if m == 0 else S0) + s]
            S_ap = acc_c[p, meta["C_DIR"] + (0 if m == 0 else S0) + s]
            corr = np.maximum(x - dq[a], 0.0).sum()
            pair_part += Bf * x - S_ap - corr
    lt = (self_part + pair_part) / (Bf + 1e-8)

    # ---- focal + label smoothing (host tail) ----
    pred, lab = prep["pred"], prep["lab"]
    se = np.empty(B)
    for c in range(N_CORES):
        for m in range(RT):
            se[c * R + m * 128:c * R + (m + 1) * 128] = \
                accs[c, :, meta["C_SE"] + m]
    lse = np.log(se)
    ptgt = pred[np.arange(B), lab]
    spred = pred.sum(axis=1)
    ce = lse - ptgt
    pt = np.exp(-ce)
    lf = (ALPHA * (1.0 - pt) ** GAMMA * ce).mean()
    ls = (-(OFF * (spred - C * lse)
            + (1.0 - SMOOTHING - OFF) * (ptgt - lse))).mean()

    total = (W_CONTRASTIVE * lc + W_TRIPLET * lt
             + W_FOCAL * lf + W_LABEL_SMOOTH * ls)
    return np.array([lc, lt, lf, ls, total], dtype=np.float32)


def kernel(pred, target, features):
    in_maps, prep = _host_prep(pred, target, features)
    nc, meta = _build(prep["S0"], prep["S1"])
    res = run_bass_kernel_spmd(nc, in_maps, core_ids=list(range(N_CORES)))
    return _combine(res.results, meta, prep)


if __name__ == "__main__":
    import reference

    inputs = reference.setup_inputs()
    expected = np.asarray(reference.reference(**inputs))
    actual = kernel(**{k: np.asarray(v) for k, v in inputs.items()})
    err = np.abs(actual - expected) / np.maximum(np.abs(expected), 1e-12)
    print("expected:", expected)
    print("actual:  ", actual)
    print("rel err: ", err)
